# revision 1
# baseline (speedup 1.0000x reference)
"""Batched ChebConv (K=3) Trainium2 kernel.

Strategy (dst-node sharding, 8 cores):
  - Nodes padded to 10240 = 80 windows x 128. Core c owns windows
    [10c, 10c+10) = nodes [1280c, 1280c+1280), all B=8 batches.
  - All batches ride in the free dim: gather rows are [512] f32 (2KB).
  - Propagation P(h)[col] += norm_e * h[row]:
      host sorts edges by destination window; per 128-edge chunk the
      vector engine builds a one-hot scatter matrix S[e, dst_local] =
      norm_e (iota-compare against dst_local, scaled by norm), and the
      PE accumulates psum[128 dst, 512] += S.T @ gathered[128 e, 512].
      Source rows are fetched with dma_gather (SWDGE indexed gather,
      int16 indices) from HBM.
  - Launch 1: Tx1 slices for all cores -> host assembles full Tx1.
    Launch 2: gathers from Tx1, Tx2 = 2*P(Tx1) - x, then the output
    epilogue out = x@W0 + Tx1@W1 + Tx2@W2 + bias via PE transposes
    (output written d-major; host untransposes).
"""

import os
import numpy as np

NC_CORES = 8
NPW = 128  # nodes per window


# ----------------------------------------------------------------------------
# host-side prep
# ----------------------------------------------------------------------------

def _prep_edges(edge_index, edge_attr, n_nodes, n_windows):
    """Sort edges by destination window; pad each window to CH chunks of 128.

    Returns (CH, src_pad[NW, CH*128] int16, dstl_pad[NW, CH*128] f32,
    norm_pad[NW, CH*128] f32).
    """
    row = edge_index[0].astype(np.int64)
    col = edge_index[1].astype(np.int64)
    ea = edge_attr.astype(np.float64)

    deg = np.zeros(n_nodes, np.float64)
    np.add.at(deg, row, ea)
    deg = deg.astype(np.float32)
    dis = np.where(deg > 0, 1.0 / np.sqrt(deg), 0.0).astype(np.float32)
    norm = -(dis[row] * edge_attr.astype(np.float32) * dis[col])

    # sort by (window, src): window grouping is required for the scatter;
    # src-sorting within a window makes the HBM gather near-sequential.
    w_of_edge = col // NPW
    order = np.lexsort((row, w_of_edge))
    cnt = np.bincount(w_of_edge, minlength=n_windows)
    ch = int(np.ceil(cnt.max() / 128))  # chunks per window
    slots = ch * 128

    src_pad = np.zeros((n_windows, slots), np.int16)
    dstl_pad = np.zeros((n_windows, slots), np.float32)
    norm_pad = np.zeros((n_windows, slots), np.float32)
    srt_row = row[order]
    srt_col = col[order]
    srt_norm = norm[order]
    pos = np.concatenate([[0], np.cumsum(cnt)])
    for w in range(n_windows):
        e0, e1 = int(pos[w]), int(pos[w + 1])
        k = e1 - e0
        src_pad[w, :k] = srt_row[e0:e1]
        dstl_pad[w, :k] = (srt_col[e0:e1] - w * NPW).astype(np.float32)
        norm_pad[w, :k] = srt_norm[e0:e1]
    return ch, src_pad, dstl_pad, norm_pad


def _wrap16(a):
    """Element i -> [i%16, i//16], replicated to 128 partitions."""
    n = a.shape[-1]
    w = a.reshape(*a.shape[:-1], n // 16, 16)
    w = np.swapaxes(w, -1, -2)  # [..., 16, n//16]
    return np.concatenate([w] * 8, axis=-2)  # [..., 128, n//16]


def _wrap128(a):
    """Element i -> [i%128, i//128]."""
    n = a.shape[-1]
    w = a.reshape(*a.shape[:-1], n // 128, 128)
    return np.swapaxes(w, -1, -2)


# ----------------------------------------------------------------------------
# device program
# ----------------------------------------------------------------------------

def _build_prog(ch, wpc, npad, bd, epilogue, use_bf16):
    """One SPMD program: per-core propagation over `wpc` windows of `ch`
    chunks; if `epilogue`, also Tx2 and the W-projection output stage."""
    from concourse import bacc, tile, library_config
    import concourse.mybir as mybir

    f32 = mybir.dt.float32
    f32r = mybir.dt.float32r
    bf16 = mybir.dt.bfloat16
    i16 = mybir.dt.int16
    gdt = bf16 if use_bf16 else f32r  # gather payload / scatter matmul dtype
    mdt = bf16 if use_bf16 else f32  # one-hot build metadata dtype
    eq = mybir.AluOpType.is_equal
    mul = mybir.AluOpType.mult
    sub = mybir.AluOpType.subtract
    add = mybir.AluOpType.add

    GSEG = 8  # chunks per dma_gather call (1024 idxs; HW fails above ~1k)
    segs = [GSEG] * (ch // GSEG)
    if ch % GSEG:
        segs.append(ch % GSEG)
    nown = wpc * NPW  # nodes owned per core

    nc = bacc.Bacc(
        "TRN2",
        target_bir_lowering=False,
        debug=False,
        num_devices=NC_CORES,
        num_swdge_queues=2,
    )

    srcg = nc.dram_tensor("srcg", [npad, bd], gdt, kind="ExternalInput")
    idx_d = nc.dram_tensor("idx", [wpc, 128, ch * 8], i16, kind="ExternalInput")
    dst_d = nc.dram_tensor("dstl", [wpc, 128, ch], f32, kind="ExternalInput")
    nra_d = nc.dram_tensor("nra", [wpc, 128, ch], f32, kind="ExternalInput")
    iota_d = nc.dram_tensor("iota", [128, 128], mdt, kind="ExternalInput")
    if epilogue:
        ident_d = nc.dram_tensor("ident", [128, 128], f32, kind="ExternalInput")
        xown_d = nc.dram_tensor("xown", [nown, bd], f32, kind="ExternalInput")
        t1own_d = nc.dram_tensor("t1own", [nown, bd], f32, kind="ExternalInput")
        w_d = nc.dram_tensor("w", [3, 64, 64], f32r, kind="ExternalInput")
        bias_d = nc.dram_tensor("bias", [64, 1], f32, kind="ExternalInput")
        outt_d = nc.dram_tensor("outt", [wpc, 64, 1024], f32, kind="ExternalOutput")
    else:
        tx1_d = nc.dram_tensor("tx1", [nown, bd], f32, kind="ExternalOutput")

    with tile.TileContext(nc) as tc:
        nc.gpsimd.load_library(library_config.mlp)
        with (
            tc.tile_pool(name="const", bufs=1) as constp,
            tc.tile_pool(name="gat", bufs=6) as gatp,
            tc.tile_pool(name="gatr", bufs=3) as gatrp,
            tc.tile_pool(name="meta", bufs=4) as metap,
            tc.tile_pool(name="oh", bufs=6) as ohp,
            tc.tile_pool(name="outp", bufs=2) as outp,
            tc.tile_pool(name="ps", bufs=2 if epilogue else 4, space="PSUM") as psp,
            tc.tile_pool(name="tps", bufs=2, space="PSUM") as tpsp,
            tc.tile_pool(name="ops", bufs=1, space="PSUM") as opsp,
        ):
            iota_t = constp.tile([128, 128], mdt, tag="iota")
            nc.sync.dma_start(iota_t[:], iota_d[:])
            if epilogue:
                ident_t = constp.tile([128, 128], f32, tag="ident")
                nc.sync.dma_start(ident_t[:], ident_d[:])
                w_t = constp.tile([64, 3, 64], f32r, tag="w")
                nc.sync.dma_start(w_t[:], w_d.ap().rearrange("k d e -> d k e"))
                bias_t = constp.tile([64, 1], f32, tag="bias")
                nc.sync.dma_start(bias_t[:], bias_d[:])

            for w in range(wpc):
                idx_t = metap.tile([128, ch * 8], i16, tag="idx")
                nc.sync.dma_start(idx_t[:], idx_d[w])
                dst_t = metap.tile([128, ch], f32, tag="dst")
                nc.sync.dma_start(dst_t[:], dst_d[w])
                nra_t = metap.tile([128, ch], f32, tag="nra")
                nc.sync.dma_start(nra_t[:], nra_d[w])

                # One-hot scatter matrices for the whole window in two
                # batched DVE tensor_tensor ops (1x mode - no 2-port perf
                # mode, so no DVE<->GpSimd port-lock against SWDGE
                # descriptor generation):
                #   S'[p, c, f] = (iota[f] == dst[p, c]) * |nrm[p, c]|
                # The sign of norm is folded into downstream constants
                # (psum accumulates -P).
                s_all = ohp.tile([128, ch, 128], gdt, tag="s")
                iota_b = (
                    iota_t[:]
                    .rearrange("p (o f) -> p o f", o=1)
                    .broadcast_to([128, ch, 128])
                )
                dst_b = (
                    dst_t[:]
                    .rearrange("p (c o) -> p c o", o=1)
                    .broadcast_to([128, ch, 128])
                )
                nra_b = (
                    nra_t[:]
                    .rearrange("p (c o) -> p c o", o=1)
                    .broadcast_to([128, ch, 128])
                )
                nc.vector.tensor_tensor(s_all[:], iota_b, dst_b, op=eq)
                nc.vector.tensor_tensor(s_all[:], s_all[:], nra_b, op=mul)
                g_ts = []
                c0 = 0
                for seg in segs:
                    pool = gatp if seg == GSEG else gatrp
                    g_t = pool.tile(
                        [128, seg, bd], gdt, tag="g" if seg == GSEG else "gr"
                    )
                    nc.gpsimd.dma_gather(
                        g_t[:],
                        srcg.ap(),
                        idx_t[:, c0 * 8 : (c0 + seg) * 8],
                        seg * 128,
                        seg * 128,
                        bd,
                        queue_num=len(g_ts) % 2,
                    )
                    g_ts.append(g_t)
                    c0 += seg
                ps = psp.tile([128, bd], f32, tag="acc")
                for c in range(ch):
                    h, cc = divmod(c, GSEG)
                    nc.tensor.matmul(
                        ps[:],
                        s_all[:, c, :],
                        g_ts[h][:, cc, :],
                        start=(c == 0),
                        stop=(c == ch - 1),
                    )

                if not epilogue:
                    o_t = outp.tile([128, bd], f32, tag="o")
                    nc.vector.tensor_scalar(o_t[:], ps[:], -1.0, None, op0=mul)
                    nc.sync.dma_start(tx1_d[w * NPW : (w + 1) * NPW, :], o_t[:])
                else:
                    xw = outp.tile([128, bd], f32, tag="xw")
                    nc.sync.dma_start(xw[:], xown_d[w * NPW : (w + 1) * NPW, :])
                    t1w = outp.tile([128, bd], f32, tag="t1w")
                    nc.sync.dma_start(t1w[:], t1own_d[w * NPW : (w + 1) * NPW, :])
                    t2w = outp.tile([128, bd], f32, tag="t2w")
                    # Tx2 = 2*P(Tx1) - x
                    nc.vector.tensor_scalar(t2w[:], ps[:], -2.0, None, op0=mul)
                    nc.vector.tensor_tensor(t2w[:], t2w[:], xw[:], op=sub)

                    # transpose all (k, b) tiles into PSUM, one big copy to
                    # SBUF, then per-quad N=512 f32r matmuls (f32r needs
                    # moving dim >= 256 for full speed)
                    ops = opsp.tile([64, 1024], f32, tag="ot")
                    tsb = outp.tile([64, 3, 1024], f32r, tag="tsb")
                    for k, src_t in enumerate((xw, t1w, t2w)):
                        tps = tpsp.tile([64, 1024], f32, tag="tp")
                        for b in range(8):
                            nc.tensor.transpose(
                                tps[:, b * 128 : (b + 1) * 128],
                                src_t[:, b * 64 : (b + 1) * 64],
                                ident_t[:],
                            )
                        nc.scalar.copy(tsb[:, k, :], tps[:])
                    for q in range(2):
                        for k in range(3):
                            nc.tensor.matmul(
                                ops[:, q * 512 : (q + 1) * 512],
                                w_t[:, k, :],
                                tsb[:, k, q * 512 : (q + 1) * 512],
                                start=(k == 0),
                                stop=(k == 2),
                            )
                    osb = outp.tile([64, 1024], f32, tag="osb")
                    nc.vector.tensor_scalar(osb[:], ops[:], bias_t[:, 0:1], None, op0=add)
                    nc.sync.dma_start(outt_d[w], osb[:])
    nc.compile()
    return nc


# ----------------------------------------------------------------------------
# entry point
# ----------------------------------------------------------------------------

LAST_EXEC_NS = []


_LAUNCH_NO = [0]


def _launch(nc, in_maps, trace):
    from concourse.bass_utils import run_bass_kernel_spmd

    tmpdir = None
    base = os.environ.get("CHEB_TMPDIR")
    if base:
        _LAUNCH_NO[0] += 1
        tmpdir = os.path.join(base, f"l{_LAUNCH_NO[0]}")
        os.makedirs(tmpdir, exist_ok=True)
    return run_bass_kernel_spmd(
        nc, in_maps, list(range(len(in_maps))), trace=trace, tmpdir=tmpdir
    )


def kernel(x, edge_index, edge_attr, W, bias):
    import ml_dtypes

    trace = bool(int(os.environ.get("CHEB_TRACE", "0")))
    use_bf16 = bool(int(os.environ.get("CHEB_BF16", "1")))
    mnp = ml_dtypes.bfloat16 if use_bf16 else np.float32

    B, N, D = x.shape
    bd = B * D
    nw = -(-N // NPW)  # windows for real nodes
    nw = -(-nw // NC_CORES) * NC_CORES  # pad to multiple of cores
    wpc = nw // NC_CORES
    npad = nw * NPW
    nown = wpc * NPW

    ch, src_pad, dstl_pad, norm_pad = _prep_edges(edge_index, edge_attr, N, nw)

    # gather source: node-major, all batches contiguous
    xg = np.zeros((npad, bd), np.float32)
    xg[:N] = np.ascontiguousarray(x.transpose(1, 0, 2)).reshape(N, bd)

    idx_all = _wrap16(src_pad)  # [nw, 128, ch*8]
    dst_all = _wrap128(dstl_pad)  # [nw, 128, ch] f32
    nra_all = -_wrap128(norm_pad)  # |norm| (norm <= 0)

    iota = np.broadcast_to(np.arange(128, dtype=np.float32), (128, 128)).astype(mnp)
    ident = np.eye(128, dtype=np.float32)

    core_ids = list(range(NC_CORES))

    # ---- launch 1: Tx1 = P(x) ----
    prog1 = _build_prog(ch, wpc, npad, bd, epilogue=False, use_bf16=use_bf16)
    xg_g = xg.astype(mnp)
    in_maps1 = []
    for c in core_ids:
        ws = slice(c * wpc, (c + 1) * wpc)
        in_maps1.append(
            {
                "srcg": xg_g,
                "idx": np.ascontiguousarray(idx_all[ws]),
                "dstl": np.ascontiguousarray(dst_all[ws]),
                "nra": np.ascontiguousarray(nra_all[ws]),
                "iota": iota,
            }
        )
    r1 = _launch(prog1, in_maps1, trace)
    tx1 = np.concatenate([r1.results[c]["tx1"] for c in core_ids], axis=0)

    # ---- launch 2: Tx2 + projection epilogue ----
    prog2 = _build_prog(ch, wpc, npad, bd, epilogue=True, use_bf16=use_bf16)
    tx1_g = tx1.astype(mnp)
    in_maps2 = []
    for c in core_ids:
        ws = slice(c * wpc, (c + 1) * wpc)
        rs = slice(c * nown, (c + 1) * nown)
        in_maps2.append(
            {
                "srcg": tx1_g,
                "idx": np.ascontiguousarray(idx_all[ws]),
                "dstl": np.ascontiguousarray(dst_all[ws]),
                "nra": np.ascontiguousarray(nra_all[ws]),
                "iota": iota,
                "ident": ident,
                "xown": np.ascontiguousarray(xg[rs]),
                "t1own": np.ascontiguousarray(tx1[rs]),
                "w": W.astype(np.float32),
                "bias": bias.astype(np.float32).reshape(64, 1),
            }
        )
    r2 = _launch(prog2, in_maps2, trace)

    global LAST_EXEC_NS
    LAST_EXEC_NS = [r1.exec_time_ns, r2.exec_time_ns]

    # outt[w, e, b*128+nl] = out[b, core*1280 + w*128 + nl, e]
    out = np.empty((B, npad, 64), np.float32)
    for c in core_ids:
        ot = r2.results[c]["outt"].reshape(wpc, 64, 8, 128)
        # -> [b, w, nl, e]
        ot = ot.transpose(2, 0, 3, 1).reshape(B, nown, 64)
        out[:, c * nown : (c + 1) * nown, :] = ot
    return out[:, :N, :]



# revision 4
# speedup vs baseline: 1.2609x; 1.2609x over previous
"""Batched ChebConv (K=3) Trainium2 kernel.

Strategy (dst-node sharding, 8 cores):
  - Nodes padded to 10240 = 80 windows x 128. Core c owns windows
    [10c, 10c+10) = nodes [1280c, 1280c+1280), all B=8 batches.
  - All batches ride in the free dim: gather rows are [512] f32 (2KB).
  - Propagation P(h)[col] += norm_e * h[row]:
      host sorts edges by destination window; per 128-edge chunk the
      vector engine builds a one-hot scatter matrix S[e, dst_local] =
      norm_e (iota-compare against dst_local, scaled by norm), and the
      PE accumulates psum[128 dst, 512] += S.T @ gathered[128 e, 512].
      Source rows are fetched with dma_gather (SWDGE indexed gather,
      int16 indices) from HBM.
  - Launch 1: Tx1 slices for all cores -> host assembles full Tx1.
    Launch 2: gathers from Tx1, Tx2 = 2*P(Tx1) - x, then the output
    epilogue out = x@W0 + Tx1@W1 + Tx2@W2 + bias via PE transposes
    (output written d-major; host untransposes).
"""

import os
import numpy as np

NC_CORES = 8
NPW = 128  # nodes per window


# ----------------------------------------------------------------------------
# host-side prep
# ----------------------------------------------------------------------------

def _prep_edges(edge_index, edge_attr, n_nodes, n_windows):
    """Sort edges by destination window; pad each window to CH chunks of 128.

    Returns (CH, src_pad[NW, CH*128] int16, dstl_pad[NW, CH*128] f32,
    norm_pad[NW, CH*128] f32).
    """
    row = edge_index[0].astype(np.int64)
    col = edge_index[1].astype(np.int64)
    ea = edge_attr.astype(np.float64)

    deg = np.zeros(n_nodes, np.float64)
    np.add.at(deg, row, ea)
    deg = deg.astype(np.float32)
    dis = np.where(deg > 0, 1.0 / np.sqrt(deg), 0.0).astype(np.float32)
    norm = -(dis[row] * edge_attr.astype(np.float32) * dis[col])

    # sort by (window, src): window grouping is required for the scatter;
    # src-sorting within a window makes the HBM gather near-sequential.
    w_of_edge = col // NPW
    order = np.lexsort((row, w_of_edge))
    cnt = np.bincount(w_of_edge, minlength=n_windows)
    ch = int(np.ceil(cnt.max() / 128))  # chunks per window
    slots = ch * 128

    src_pad = np.zeros((n_windows, slots), np.int16)
    dstl_pad = np.zeros((n_windows, slots), np.float32)
    norm_pad = np.zeros((n_windows, slots), np.float32)
    srt_row = row[order]
    srt_col = col[order]
    srt_norm = norm[order]
    pos = np.concatenate([[0], np.cumsum(cnt)])
    for w in range(n_windows):
        e0, e1 = int(pos[w]), int(pos[w + 1])
        k = e1 - e0
        src_pad[w, :k] = srt_row[e0:e1]
        dstl_pad[w, :k] = (srt_col[e0:e1] - w * NPW).astype(np.float32)
        norm_pad[w, :k] = srt_norm[e0:e1]
    return ch, src_pad, dstl_pad, norm_pad


def _wrap16(a):
    """Element i -> [i%16, i//16], replicated to 128 partitions."""
    n = a.shape[-1]
    w = a.reshape(*a.shape[:-1], n // 16, 16)
    w = np.swapaxes(w, -1, -2)  # [..., 16, n//16]
    return np.concatenate([w] * 8, axis=-2)  # [..., 128, n//16]


def _wrap128(a):
    """Element i -> [i%128, i//128]."""
    n = a.shape[-1]
    w = a.reshape(*a.shape[:-1], n // 128, 128)
    return np.swapaxes(w, -1, -2)


# ----------------------------------------------------------------------------
# device program
# ----------------------------------------------------------------------------

def _build_prog(ch, wpc, npad, bd, epilogue, use_bf16):
    """One SPMD program: per-core propagation over `wpc` windows of `ch`
    chunks; if `epilogue`, also Tx2 and the W-projection output stage."""
    from concourse import bacc, tile, library_config
    import concourse.mybir as mybir

    f32 = mybir.dt.float32
    f32r = mybir.dt.float32r
    bf16 = mybir.dt.bfloat16
    i16 = mybir.dt.int16
    gdt = bf16 if use_bf16 else f32r  # gather payload / scatter matmul dtype
    mdt = bf16 if use_bf16 else f32  # one-hot build metadata dtype
    eq = mybir.AluOpType.is_equal
    mul = mybir.AluOpType.mult
    sub = mybir.AluOpType.subtract
    add = mybir.AluOpType.add

    GSEG = 8  # chunks per dma_gather call (1024 idxs; HW fails above ~1k)
    segs = [GSEG] * (ch // GSEG)
    if ch % GSEG:
        segs.append(ch % GSEG)
    nown = wpc * NPW  # nodes owned per core

    nc = bacc.Bacc(
        "TRN2",
        target_bir_lowering=False,
        debug=False,
        num_devices=NC_CORES,
        num_swdge_queues=4,
    )

    srcg = nc.dram_tensor("srcg", [npad, bd], gdt, kind="ExternalInput")
    idx_d = nc.dram_tensor("idx", [wpc, 128, ch * 8], i16, kind="ExternalInput")
    dst_d = nc.dram_tensor("dstl", [wpc, 128, ch], f32, kind="ExternalInput")
    nra_d = nc.dram_tensor("nra", [wpc, 128, ch], f32, kind="ExternalInput")
    iota_d = nc.dram_tensor("iota", [128, 128], mdt, kind="ExternalInput")
    if epilogue:
        ident_d = nc.dram_tensor("ident", [128, 128], f32, kind="ExternalInput")
        xown_d = nc.dram_tensor("xown", [nown, bd], f32, kind="ExternalInput")
        t1own_d = nc.dram_tensor("t1own", [nown, bd], f32, kind="ExternalInput")
        w_d = nc.dram_tensor("w", [3, 64, 64], f32r, kind="ExternalInput")
        bias_d = nc.dram_tensor("bias", [64, 1], f32, kind="ExternalInput")
        outt_d = nc.dram_tensor("outt", [wpc, 64, 1024], f32, kind="ExternalOutput")
    else:
        tx1_d = nc.dram_tensor("tx1", [nown, bd], f32, kind="ExternalOutput")

    with tile.TileContext(nc) as tc:
        nc.gpsimd.load_library(library_config.mlp)
        with (
            tc.tile_pool(name="const", bufs=1) as constp,
            tc.tile_pool(name="gat", bufs=6) as gatp,
            tc.tile_pool(name="gatr", bufs=3) as gatrp,
            tc.tile_pool(name="meta", bufs=4) as metap,
            tc.tile_pool(name="oh", bufs=6) as ohp,
            tc.tile_pool(name="outp", bufs=2) as outp,
            tc.tile_pool(name="ps", bufs=2 if epilogue else 4, space="PSUM") as psp,
            tc.tile_pool(name="tps", bufs=2, space="PSUM") as tpsp,
            tc.tile_pool(name="ops", bufs=1, space="PSUM") as opsp,
        ):
            gq = [0]  # global gather-call counter for queue round-robin
            iota_t = constp.tile([128, 128], mdt, tag="iota")
            nc.sync.dma_start(iota_t[:], iota_d[:])
            if epilogue:
                ident_t = constp.tile([128, 128], f32, tag="ident")
                nc.sync.dma_start(ident_t[:], ident_d[:])
                w_t = constp.tile([64, 3, 64], f32r, tag="w")
                nc.sync.dma_start(w_t[:], w_d.ap().rearrange("k d e -> d k e"))
                bias_t = constp.tile([64, 1], f32, tag="bias")
                nc.sync.dma_start(bias_t[:], bias_d[:])

            for w in range(wpc):
                idx_t = metap.tile([128, ch * 8], i16, tag="idx")
                nc.sync.dma_start(idx_t[:], idx_d[w])
                dst_t = metap.tile([128, ch], f32, tag="dst")
                nc.sync.dma_start(dst_t[:], dst_d[w])
                nra_t = metap.tile([128, ch], f32, tag="nra")
                nc.sync.dma_start(nra_t[:], nra_d[w])

                # One-hot scatter matrices for the whole window in two
                # batched DVE tensor_tensor ops (1x mode - no 2-port perf
                # mode, so no DVE<->GpSimd port-lock against SWDGE
                # descriptor generation):
                #   S'[p, c, f] = (iota[f] == dst[p, c]) * |nrm[p, c]|
                # The sign of norm is folded into downstream constants
                # (psum accumulates -P).
                s_all = ohp.tile([128, ch, 128], gdt, tag="s")
                iota_b = (
                    iota_t[:]
                    .rearrange("p (o f) -> p o f", o=1)
                    .broadcast_to([128, ch, 128])
                )
                dst_b = (
                    dst_t[:]
                    .rearrange("p (c o) -> p c o", o=1)
                    .broadcast_to([128, ch, 128])
                )
                nra_b = (
                    nra_t[:]
                    .rearrange("p (c o) -> p c o", o=1)
                    .broadcast_to([128, ch, 128])
                )
                nc.vector.tensor_tensor(s_all[:], iota_b, dst_b, op=eq)
                nc.vector.tensor_tensor(s_all[:], s_all[:], nra_b, op=mul)
                g_ts = []
                c0 = 0
                for seg in segs:
                    pool = gatp if seg == GSEG else gatrp
                    g_t = pool.tile(
                        [128, seg, bd], gdt, tag="g" if seg == GSEG else "gr"
                    )
                    nc.gpsimd.dma_gather(
                        g_t[:],
                        srcg.ap(),
                        idx_t[:, c0 * 8 : (c0 + seg) * 8],
                        seg * 128,
                        seg * 128,
                        bd,
                        queue_num=gq[0] % 4,
                    )
                    gq[0] += 1
                    g_ts.append(g_t)
                    c0 += seg
                ps = psp.tile([128, bd], f32, tag="acc")
                for c in range(ch):
                    h, cc = divmod(c, GSEG)
                    nc.tensor.matmul(
                        ps[:],
                        s_all[:, c, :],
                        g_ts[h][:, cc, :],
                        start=(c == 0),
                        stop=(c == ch - 1),
                    )

                if not epilogue:
                    o_t = outp.tile([128, bd], f32, tag="o")
                    nc.vector.tensor_scalar(o_t[:], ps[:], -1.0, None, op0=mul)
                    nc.sync.dma_start(tx1_d[w * NPW : (w + 1) * NPW, :], o_t[:])
                else:
                    xw = outp.tile([128, bd], f32, tag="xw")
                    nc.sync.dma_start(xw[:], xown_d[w * NPW : (w + 1) * NPW, :])
                    t1w = outp.tile([128, bd], f32, tag="t1w")
                    nc.sync.dma_start(t1w[:], t1own_d[w * NPW : (w + 1) * NPW, :])
                    t2w = outp.tile([128, bd], f32, tag="t2w")
                    # Tx2 = 2*P(Tx1) - x
                    nc.vector.tensor_scalar(t2w[:], ps[:], -2.0, None, op0=mul)
                    nc.vector.tensor_tensor(t2w[:], t2w[:], xw[:], op=sub)

                    # transpose all (k, b) tiles into PSUM, one big copy to
                    # SBUF, then per-quad N=512 f32r matmuls (f32r needs
                    # moving dim >= 256 for full speed)
                    ops = opsp.tile([64, 1024], f32, tag="ot")
                    tsb = outp.tile([64, 3, 1024], f32r, tag="tsb")
                    for k, src_t in enumerate((xw, t1w, t2w)):
                        tps = tpsp.tile([64, 1024], f32, tag="tp")
                        for b in range(8):
                            nc.tensor.transpose(
                                tps[:, b * 128 : (b + 1) * 128],
                                src_t[:, b * 64 : (b + 1) * 64],
                                ident_t[:],
                            )
                        nc.scalar.copy(tsb[:, k, :], tps[:])
                    for q in range(2):
                        for k in range(3):
                            nc.tensor.matmul(
                                ops[:, q * 512 : (q + 1) * 512],
                                w_t[:, k, :],
                                tsb[:, k, q * 512 : (q + 1) * 512],
                                start=(k == 0),
                                stop=(k == 2),
                            )
                    osb = outp.tile([64, 1024], f32, tag="osb")
                    nc.vector.tensor_scalar(osb[:], ops[:], bias_t[:, 0:1], None, op0=add)
                    nc.sync.dma_start(outt_d[w], osb[:])
    nc.compile()
    return nc


# ----------------------------------------------------------------------------
# entry point
# ----------------------------------------------------------------------------

LAST_EXEC_NS = []


_LAUNCH_NO = [0]


def _launch(nc, in_maps, trace):
    from concourse.bass_utils import run_bass_kernel_spmd

    tmpdir = None
    base = os.environ.get("CHEB_TMPDIR")
    if base:
        _LAUNCH_NO[0] += 1
        tmpdir = os.path.join(base, f"l{_LAUNCH_NO[0]}")
        os.makedirs(tmpdir, exist_ok=True)
    return run_bass_kernel_spmd(
        nc, in_maps, list(range(len(in_maps))), trace=trace, tmpdir=tmpdir
    )


def kernel(x, edge_index, edge_attr, W, bias):
    import ml_dtypes

    trace = bool(int(os.environ.get("CHEB_TRACE", "0")))
    use_bf16 = bool(int(os.environ.get("CHEB_BF16", "1")))
    mnp = ml_dtypes.bfloat16 if use_bf16 else np.float32

    B, N, D = x.shape
    bd = B * D
    nw = -(-N // NPW)  # windows for real nodes
    nw = -(-nw // NC_CORES) * NC_CORES  # pad to multiple of cores
    wpc = nw // NC_CORES
    npad = nw * NPW
    nown = wpc * NPW

    ch, src_pad, dstl_pad, norm_pad = _prep_edges(edge_index, edge_attr, N, nw)

    # gather source: node-major, all batches contiguous
    xg = np.zeros((npad, bd), np.float32)
    xg[:N] = np.ascontiguousarray(x.transpose(1, 0, 2)).reshape(N, bd)

    idx_all = _wrap16(src_pad)  # [nw, 128, ch*8]
    dst_all = _wrap128(dstl_pad)  # [nw, 128, ch] f32
    nra_all = -_wrap128(norm_pad)  # |norm| (norm <= 0)

    iota = np.broadcast_to(np.arange(128, dtype=np.float32), (128, 128)).astype(mnp)
    ident = np.eye(128, dtype=np.float32)

    core_ids = list(range(NC_CORES))

    # ---- launch 1: Tx1 = P(x) ----
    prog1 = _build_prog(ch, wpc, npad, bd, epilogue=False, use_bf16=use_bf16)
    xg_g = xg.astype(mnp)
    in_maps1 = []
    for c in core_ids:
        ws = slice(c * wpc, (c + 1) * wpc)
        in_maps1.append(
            {
                "srcg": xg_g,
                "idx": np.ascontiguousarray(idx_all[ws]),
                "dstl": np.ascontiguousarray(dst_all[ws]),
                "nra": np.ascontiguousarray(nra_all[ws]),
                "iota": iota,
            }
        )
    r1 = _launch(prog1, in_maps1, trace)
    tx1 = np.concatenate([r1.results[c]["tx1"] for c in core_ids], axis=0)

    # ---- launch 2: Tx2 + projection epilogue ----
    prog2 = _build_prog(ch, wpc, npad, bd, epilogue=True, use_bf16=use_bf16)
    tx1_g = tx1.astype(mnp)
    in_maps2 = []
    for c in core_ids:
        ws = slice(c * wpc, (c + 1) * wpc)
        rs = slice(c * nown, (c + 1) * nown)
        in_maps2.append(
            {
                "srcg": tx1_g,
                "idx": np.ascontiguousarray(idx_all[ws]),
                "dstl": np.ascontiguousarray(dst_all[ws]),
                "nra": np.ascontiguousarray(nra_all[ws]),
                "iota": iota,
                "ident": ident,
                "xown": np.ascontiguousarray(xg[rs]),
                "t1own": np.ascontiguousarray(tx1[rs]),
                "w": W.astype(np.float32),
                "bias": bias.astype(np.float32).reshape(64, 1),
            }
        )
    r2 = _launch(prog2, in_maps2, trace)

    global LAST_EXEC_NS
    LAST_EXEC_NS = [r1.exec_time_ns, r2.exec_time_ns]

    # outt[w, e, b*128+nl] = out[b, core*1280 + w*128 + nl, e]
    out = np.empty((B, npad, 64), np.float32)
    for c in core_ids:
        ot = r2.results[c]["outt"].reshape(wpc, 64, 8, 128)
        # -> [b, w, nl, e]
        ot = ot.transpose(2, 0, 3, 1).reshape(B, nown, 64)
        out[:, c * nown : (c + 1) * nown, :] = ot
    return out[:, :N, :]



# revision 7
# speedup vs baseline: 1.3129x; 1.0412x over previous
"""Batched ChebConv (K=3) Trainium2 kernel.

Strategy (dst-node sharding, 8 cores, 2 launches):
  out = x@W0 + Tx1@W1 + Tx2@W2,  Tx1 = P(x),  Tx2 = 2*P(Tx1) - x
      = x@(W0-W2) + Tx1@W1 + 2*P(Tx1@W2)        [P commutes with W]

  All feature math runs in the transposed domain (features in partitions),
  so the host supplies x^T tiles and the device never transposes x:
    out^T = (W0-W2)^T x^T + W1^T Tx1^T + 2*P(z)^T,   z = Tx1@W2.

  Launch 1: per dst window, scatter-matmul propagation psum = -P(x)
    (edge one-hot S built by DVE, per-edge source rows fetched by SWDGE
    dma_gather, 4 queues round-robin), then 8 PE transposes of Tx1 and
    projections zT = W2^T Tx1^T and outP = (W0-W2)^T x^T + W1^T Tx1^T
    + bias.  Outputs (bf16): zT, outP.  Host relayouts zT -> node-major
    z table.
  Launch 2: same propagation on z, then out^T = outP + 2*P(z)^T.

  Windows are assigned to (core, slot) by sorted edge count so every
  core's slot j has a similar chunk count ch[j] (compile-time shared
  SPMD shape, minimal padding).
"""

import os
import numpy as np

NC_CORES = 8
NPW = 128  # nodes per window
GSEG = 8  # max chunks per dma_gather call (1024 idxs; HW fails above ~1k)


# ----------------------------------------------------------------------------
# host-side prep
# ----------------------------------------------------------------------------

def _prep_edges(edge_index, edge_attr, n_nodes, n_windows):
    """Sort edges by destination window, then source.

    Returns (cnt[nw], srt_row, srt_col, srt_norm) with per-window slices
    given by cumsum(cnt).
    """
    row = edge_index[0].astype(np.int64)
    col = edge_index[1].astype(np.int64)
    ea = edge_attr.astype(np.float64)

    deg = np.zeros(n_nodes, np.float64)
    np.add.at(deg, row, ea)
    deg = deg.astype(np.float32)
    dis = np.where(deg > 0, 1.0 / np.sqrt(deg), 0.0).astype(np.float32)
    norm = -(dis[row] * edge_attr.astype(np.float32) * dis[col])

    w_of_edge = col // NPW
    order = np.lexsort((row, w_of_edge))
    cnt = np.bincount(w_of_edge, minlength=n_windows)
    return cnt, row[order], col[order], norm[order]


def _wrap16(a):
    """Element i -> [i%16, i//16], replicated to 128 partitions."""
    n = a.shape[-1]
    w = a.reshape(*a.shape[:-1], n // 16, 16)
    w = np.swapaxes(w, -1, -2)  # [..., 16, n//16]
    return np.concatenate([w] * 8, axis=-2)  # [..., 128, n//16]


def _wrap128(a):
    """Element i -> [i%128, i//128]."""
    n = a.shape[-1]
    w = a.reshape(*a.shape[:-1], n // 128, 128)
    return np.swapaxes(w, -1, -2)


# ----------------------------------------------------------------------------
# device program
# ----------------------------------------------------------------------------

def _build_prog(chs, npad, bd, phase2):
    """One SPMD program over wpc = len(chs) window slots; slot j has chs[j]
    128-edge chunks.  phase2 selects the combine epilogue."""
    from concourse import bacc, tile, library_config
    import concourse.mybir as mybir

    f32 = mybir.dt.float32
    bf16 = mybir.dt.bfloat16
    i16 = mybir.dt.int16
    eq = mybir.AluOpType.is_equal
    mul = mybir.AluOpType.mult
    add = mybir.AluOpType.add
    copy_f = mybir.ActivationFunctionType.Copy

    wpc = len(chs)
    CH = int(sum(chs))
    off = np.concatenate([[0], np.cumsum(chs)]).astype(int)

    nc = bacc.Bacc(
        "TRN2",
        target_bir_lowering=False,
        debug=False,
        num_devices=NC_CORES,
        num_swdge_queues=4,
    )

    srcg = nc.dram_tensor("srcg", [npad, bd], bf16, kind="ExternalInput")
    idx_d = nc.dram_tensor("idx", [128, CH * 8], i16, kind="ExternalInput")
    dst_d = nc.dram_tensor("dstl", [128, CH], f32, kind="ExternalInput")
    nra_d = nc.dram_tensor("nra", [128, CH], f32, kind="ExternalInput")
    iota_d = nc.dram_tensor("iota", [128, 128], bf16, kind="ExternalInput")
    ident_d = nc.dram_tensor("ident", [128, 128], bf16, kind="ExternalInput")
    if phase2:
        outp_d = nc.dram_tensor("outp", [wpc, 64, 1024], bf16, kind="ExternalInput")
        outt_d = nc.dram_tensor("outt", [wpc, 64, 1024], bf16, kind="ExternalOutput")
    else:
        xt_d = nc.dram_tensor("xt", [wpc, 64, 1024], bf16, kind="ExternalInput")
        w1_d = nc.dram_tensor("w1", [64, 64], bf16, kind="ExternalInput")
        w2_d = nc.dram_tensor("w2", [64, 64], bf16, kind="ExternalInput")
        w02_d = nc.dram_tensor("w02", [64, 64], bf16, kind="ExternalInput")
        bias_d = nc.dram_tensor("bias", [64, 1], f32, kind="ExternalInput")
        zt_d = nc.dram_tensor("zt", [wpc, 64, 1024], bf16, kind="ExternalOutput")
        outp_d = nc.dram_tensor("outp", [wpc, 64, 1024], bf16, kind="ExternalOutput")

    with tile.TileContext(nc) as tc:
        nc.gpsimd.load_library(library_config.mlp)
        with (
            tc.tile_pool(name="const", bufs=1) as constp,
            tc.tile_pool(name="gat", bufs=7) as gatp,
            tc.tile_pool(name="meta", bufs=4) as metap,
            tc.tile_pool(name="oh", bufs=4) as ohp,
            tc.tile_pool(name="sb", bufs=3) as sbp,
            tc.tile_pool(name="out", bufs=3) as outp_pool,
            tc.tile_pool(name="ps", bufs=3 if phase2 else 2, space="PSUM") as psp,
            tc.tile_pool(name="tps", bufs=2 if phase2 else 1, space="PSUM") as tpsp,
            tc.tile_pool(name="ops", bufs=1, space="PSUM") as opsp,
        ):
            gq = [0]  # global gather-call counter for queue round-robin
            iota_t = constp.tile([128, 128], bf16, tag="iota")
            nc.sync.dma_start(iota_t[:], iota_d[:])
            ident_t = constp.tile([128, 128], bf16, tag="ident")
            nc.sync.dma_start(ident_t[:], ident_d[:])
            if not phase2:
                w1_t = constp.tile([64, 64], bf16, tag="w1")
                nc.sync.dma_start(w1_t[:], w1_d[:])
                w2_t = constp.tile([64, 64], bf16, tag="w2")
                nc.sync.dma_start(w2_t[:], w2_d[:])
                w02_t = constp.tile([64, 64], bf16, tag="w02")
                nc.sync.dma_start(w02_t[:], w02_d[:])
                bias_t = constp.tile([64, 1], f32, tag="bias")
                nc.sync.dma_start(bias_t[:], bias_d[:])

            for j in range(wpc):
                ch = int(chs[j])
                c0, c1 = int(off[j]), int(off[j + 1])
                idx_t = metap.tile([128, ch * 8], i16, tag="idx")
                nc.sync.dma_start(idx_t[:], idx_d[:, c0 * 8 : c1 * 8])
                dst_t = metap.tile([128, ch], f32, tag="dst")
                nc.sync.dma_start(dst_t[:], dst_d[:, c0:c1])
                nra_t = metap.tile([128, ch], f32, tag="nra")
                nc.sync.dma_start(nra_t[:], nra_d[:, c0:c1])
                if phase2:
                    outp_t = outp_pool.tile([64, 1024], bf16, tag="outp")
                    nc.sync.dma_start(outp_t[:], outp_d[j])
                else:
                    xt_t = outp_pool.tile([64, 1024], bf16, tag="xt")
                    nc.sync.dma_start(xt_t[:], xt_d[j])

                # One-hot scatter matrices for the whole window, two batched
                # DVE ops: S[p, c, f] = (iota[f] == dst[p, c]) * |nrm[p, c]|.
                # Sign of norm folded into the epilogue scales.
                s_all = ohp.tile([128, ch, 128], bf16, tag="s")
                iota_b = (
                    iota_t[:]
                    .rearrange("p (o f) -> p o f", o=1)
                    .broadcast_to([128, ch, 128])
                )
                dst_b = (
                    dst_t[:]
                    .rearrange("p (c o) -> p c o", o=1)
                    .broadcast_to([128, ch, 128])
                )
                nra_b = (
                    nra_t[:]
                    .rearrange("p (c o) -> p c o", o=1)
                    .broadcast_to([128, ch, 128])
                )
                nc.vector.tensor_tensor(s_all[:], iota_b, dst_b, op=eq)
                nc.vector.tensor_tensor(s_all[:], s_all[:], nra_b, op=mul)

                # per-edge source rows via SWDGE gather, balanced calls
                ncalls = -(-ch // GSEG)
                base, rem = divmod(ch, ncalls)
                segs = [base + (k < rem) for k in range(ncalls)]
                g_ts = []
                s0 = 0
                for seg in segs:
                    g_t = gatp.tile([128, GSEG, bd], bf16, tag="g")
                    nc.gpsimd.dma_gather(
                        g_t[:, :seg, :],
                        srcg.ap(),
                        idx_t[:, s0 * 8 : (s0 + seg) * 8],
                        seg * 128,
                        seg * 128,
                        bd,
                        queue_num=gq[0] % 4,
                    )
                    gq[0] += 1
                    g_ts.append((g_t, s0, seg))
                    s0 += seg

                # psum[dst, bf] -= sum_e norm_e * src_row_e  (S holds |norm|)
                ps = psp.tile([128, bd], f32, tag="acc")
                c = 0
                for g_t, s0, seg in g_ts:
                    for cc in range(seg):
                        nc.tensor.matmul(
                            ps[:],
                            s_all[:, c, :],
                            g_t[:, cc, :],
                            start=(c == 0),
                            stop=(c == ch - 1),
                        )
                        c += 1

                # h_sb = scale * psum  (scale -1 -> Tx1;  -2 -> 2*P(z))
                h_sb = sbp.tile([128, bd], bf16, tag="h")
                nc.scalar.activation(
                    h_sb[:], ps[:], copy_f, scale=-2.0 if phase2 else -1.0
                )
                # 8 transposes -> tps[64, 1024] = h^T
                tps = tpsp.tile([64, 1024], bf16, tag="tp")
                for b in range(8):
                    nc.tensor.transpose(
                        tps[:, b * 128 : (b + 1) * 128],
                        h_sb[:, b * 64 : (b + 1) * 64],
                        ident_t[:],
                    )

                if phase2:
                    # out^T = outP + 2*P(z)^T
                    o_sb = outp_pool.tile([64, 1024], bf16, tag="o")
                    nc.vector.tensor_tensor(o_sb[:], tps[:], outp_t[:], op=add)
                    nc.sync.dma_start(outt_d[j], o_sb[:])
                else:
                    t1t = sbp.tile([64, 1024], bf16, tag="t1t")
                    nc.scalar.copy(t1t[:], tps[:])
                    # zT = W2^T Tx1^T
                    zps = opsp.tile([64, 1024], f32, tag="zps")
                    for q in range(2):
                        nc.tensor.matmul(
                            zps[:, q * 512 : (q + 1) * 512],
                            w2_t[:],
                            t1t[:, q * 512 : (q + 1) * 512],
                            start=True,
                            stop=True,
                        )
                    z_sb = sbp.tile([64, 1024], bf16, tag="z")
                    nc.scalar.copy(z_sb[:], zps[:])
                    nc.sync.dma_start(zt_d[j], z_sb[:])
                    # outP = (W0-W2)^T x^T + W1^T Tx1^T + bias
                    ops = opsp.tile([64, 1024], f32, tag="ops")
                    for q in range(2):
                        nc.tensor.matmul(
                            ops[:, q * 512 : (q + 1) * 512],
                            w02_t[:],
                            xt_t[:, q * 512 : (q + 1) * 512],
                            start=True,
                            stop=False,
                        )
                        nc.tensor.matmul(
                            ops[:, q * 512 : (q + 1) * 512],
                            w1_t[:],
                            t1t[:, q * 512 : (q + 1) * 512],
                            start=False,
                            stop=True,
                        )
                    op_sb = outp_pool.tile([64, 1024], bf16, tag="opsb")
                    nc.vector.tensor_scalar(
                        op_sb[:], ops[:], bias_t[:, 0:1], None, op0=add
                    )
                    nc.sync.dma_start(outp_d[j], op_sb[:])
    nc.compile()
    return nc


# ----------------------------------------------------------------------------
# entry point
# ----------------------------------------------------------------------------

LAST_EXEC_NS = []
_LAUNCH_NO = [0]


def _launch(nc, in_maps, trace):
    from concourse.bass_utils import run_bass_kernel_spmd

    tmpdir = None
    base = os.environ.get("CHEB_TMPDIR")
    if base:
        _LAUNCH_NO[0] += 1
        tmpdir = os.path.join(base, f"l{_LAUNCH_NO[0]}")
        os.makedirs(tmpdir, exist_ok=True)
    return run_bass_kernel_spmd(
        nc, in_maps, list(range(len(in_maps))), trace=trace, tmpdir=tmpdir
    )


def kernel(x, edge_index, edge_attr, W, bias):
    import ml_dtypes

    bf = ml_dtypes.bfloat16
    trace = bool(int(os.environ.get("CHEB_TRACE", "0")))

    B, N, D = x.shape
    bd = B * D
    nw = -(-N // NPW)  # windows for real nodes
    nw = -(-nw // NC_CORES) * NC_CORES  # pad to multiple of cores
    wpc = nw // NC_CORES
    npad = nw * NPW

    cnt, srt_row, srt_col, srt_norm = _prep_edges(edge_index, edge_attr, N, nw)
    pos = np.concatenate([[0], np.cumsum(cnt)]).astype(int)

    # window -> (slot, core) by descending edge count: slot j's chunk count is
    # shared across cores with minimal padding.
    order = np.argsort(-cnt, kind="stable")
    wins = order.reshape(wpc, NC_CORES)  # wins[j, c] = window of core c, slot j
    chs = [int(-(-cnt[wins[j, 0]] // 128)) for j in range(wpc)]
    CH = int(sum(chs))
    off = np.concatenate([[0], np.cumsum(chs)]).astype(int)

    # per-(core, slot) metadata, flattened along chunks
    src_pad = np.zeros((NC_CORES, CH * 128), np.int16)
    dstl_pad = np.zeros((NC_CORES, CH * 128), np.float32)
    norm_pad = np.zeros((NC_CORES, CH * 128), np.float32)
    for j in range(wpc):
        for c in range(NC_CORES):
            w = int(wins[j, c])
            e0, e1 = int(pos[w]), int(pos[w + 1])
            k = e1 - e0
            s0 = int(off[j]) * 128
            src_pad[c, s0 : s0 + k] = srt_row[e0:e1]
            dstl_pad[c, s0 : s0 + k] = (srt_col[e0:e1] - w * NPW).astype(np.float32)
            norm_pad[c, s0 : s0 + k] = srt_norm[e0:e1]

    idx_all = _wrap16(src_pad)  # [cores, 128, CH*8] int16
    dst_all = _wrap128(dstl_pad)  # [cores, 128, CH]
    nra_all = -_wrap128(norm_pad)  # |norm| (norm <= 0)

    iota = np.broadcast_to(np.arange(128, dtype=np.float32), (128, 128)).astype(bf)
    ident = np.eye(128, dtype=np.float32).astype(bf)

    # gather table for launch 1: node-major, all batches contiguous
    xg = np.zeros((npad, bd), bf)
    xnb = np.ascontiguousarray(x.transpose(1, 0, 2)).reshape(N, bd)
    xg[:N] = xnb.astype(bf)

    # x^T tiles per (core, slot): [64, b*128+nl]
    xpad = np.zeros((B, npad, D), np.float32)
    xpad[:, :N] = x
    # xt_full[w, d, b, nl] = x[b, w*128+nl, d]
    xt_full = xpad.reshape(B, nw, NPW, D).transpose(1, 3, 0, 2).astype(bf)
    xt_full = np.ascontiguousarray(xt_full.reshape(nw, 64, 1024))

    W = W.astype(np.float32)
    w1 = np.ascontiguousarray(W[1]).astype(bf)
    w2 = np.ascontiguousarray(W[2]).astype(bf)
    w02 = np.ascontiguousarray(W[0] - W[2]).astype(bf)
    bias_in = bias.astype(np.float32).reshape(64, 1)

    core_ids = list(range(NC_CORES))

    # ---- launch 1 ----
    prog1 = _build_prog(chs, npad, bd, phase2=False)
    in_maps1 = []
    for c in core_ids:
        in_maps1.append(
            {
                "srcg": xg,
                "idx": np.ascontiguousarray(idx_all[c]),
                "dstl": np.ascontiguousarray(dst_all[c]),
                "nra": np.ascontiguousarray(nra_all[c]),
                "iota": iota,
                "ident": ident,
                "xt": np.ascontiguousarray(xt_full[wins[:, c]]),
                "w1": w1,
                "w2": w2,
                "w02": w02,
                "bias": bias_in,
            }
        )
    r1 = _launch(prog1, in_maps1, trace)

    # assemble z table (node-major) from zT tiles; keep outP per core
    zg = np.zeros((npad, bd), bf)
    outp_tiles = []
    for c in core_ids:
        zt = r1.results[c]["zt"]  # [wpc, 64, 1024] bf16
        outp_tiles.append(r1.results[c]["outp"])
        # z[w*128+nl, b*64+d] = zt[j, d, b*128+nl]
        z = zt.reshape(wpc, 64, 8, 128).transpose(0, 3, 2, 1)  # [j, nl, b, d]
        zg[(wins[:, c][:, None] * NPW + np.arange(NPW)[None, :]).reshape(-1)] = (
            z.reshape(wpc * NPW, bd)
        )

    # ---- launch 2 ----
    prog2 = _build_prog(chs, npad, bd, phase2=True)
    in_maps2 = []
    for c in core_ids:
        in_maps2.append(
            {
                "srcg": zg,
                "idx": np.ascontiguousarray(idx_all[c]),
                "dstl": np.ascontiguousarray(dst_all[c]),
                "nra": np.ascontiguousarray(nra_all[c]),
                "iota": iota,
                "ident": ident,
                "outp": outp_tiles[c],
            }
        )
    r2 = _launch(prog2, in_maps2, trace)

    global LAST_EXEC_NS
    LAST_EXEC_NS = [r1.exec_time_ns, r2.exec_time_ns]

    # out[b, w*128+nl, e] = outt[c][j, e, b*128+nl]
    out = np.empty((B, npad, 64), np.float32)
    for c in core_ids:
        ot = r2.results[c]["outt"].astype(np.float32)  # [wpc, 64, 1024]
        ot = ot.reshape(wpc, 64, 8, 128).transpose(2, 0, 3, 1)  # [b, j, nl, e]
        w_ids = wins[:, c]
        out[:, (w_ids[:, None] * NPW + np.arange(NPW)[None, :]).reshape(-1), :] = (
            ot.reshape(B, wpc * NPW, 64)
        )
    return out[:, :N, :]


# revision 15
# speedup vs baseline: 1.3598x; 1.0357x over previous
"""Batched ChebConv (K=3) Trainium2 kernel.

Strategy (dst-node sharding, 8 cores, 2 launches):
  out = x@W0 + Tx1@W1 + Tx2@W2,  Tx1 = P(x),  Tx2 = 2*P(Tx1) - x
      = x@(W0-W2) + Tx1@W1 + 2*P(Tx1@W2)        [P commutes with W]

  All feature math runs in the transposed domain (features in partitions),
  so the host supplies x^T tiles and the device never transposes x:
    out^T = (W0-W2)^T x^T + W1^T Tx1^T + 2*P(z)^T,   z = Tx1@W2.

  Launch 1: per dst window, scatter-matmul propagation psum = -P(x)
    (edge one-hot S built by DVE, per-edge source rows fetched by SWDGE
    dma_gather, 4 queues round-robin), then 8 PE transposes of Tx1 and
    projections zT = W2^T Tx1^T and outP = (W0-W2)^T x^T + W1^T Tx1^T
    + bias.  Outputs (bf16): zT, outP.  Host relayouts zT -> node-major
    z table.
  Launch 2: same propagation on z, then out^T = outP + 2*P(z)^T.

  Windows are assigned to (core, slot) by sorted edge count so every
  core's slot j has a similar chunk count ch[j] (compile-time shared
  SPMD shape, minimal padding).
"""

import os
import numpy as np

NC_CORES = 8
NPW = 128  # nodes per window
GSEG = 8  # max chunks per dma_gather call (1024 idxs; HW fails above ~1k)


# ----------------------------------------------------------------------------
# host-side prep
# ----------------------------------------------------------------------------

def _prep_edges(edge_index, edge_attr, n_nodes, n_windows):
    """Sort edges by destination window, then source.

    Returns (cnt[nw], srt_row, srt_col, srt_norm) with per-window slices
    given by cumsum(cnt).
    """
    row = edge_index[0].astype(np.int64)
    col = edge_index[1].astype(np.int64)
    ea = edge_attr.astype(np.float64)

    deg = np.zeros(n_nodes, np.float64)
    np.add.at(deg, row, ea)
    deg = deg.astype(np.float32)
    dis = np.where(deg > 0, 1.0 / np.sqrt(deg), 0.0).astype(np.float32)
    norm = -(dis[row] * edge_attr.astype(np.float32) * dis[col])

    w_of_edge = col // NPW
    order = np.lexsort((row, w_of_edge))
    cnt = np.bincount(w_of_edge, minlength=n_windows)
    return cnt, row[order], col[order], norm[order]


def _wrap16(a):
    """Element i -> [i%16, i//16], replicated to 128 partitions."""
    n = a.shape[-1]
    w = a.reshape(*a.shape[:-1], n // 16, 16)
    w = np.swapaxes(w, -1, -2)  # [..., 16, n//16]
    return np.concatenate([w] * 8, axis=-2)  # [..., 128, n//16]


def _wrap128(a):
    """Element i -> [i%128, i//128]."""
    n = a.shape[-1]
    w = a.reshape(*a.shape[:-1], n // 128, 128)
    return np.swapaxes(w, -1, -2)


# ----------------------------------------------------------------------------
# device program
# ----------------------------------------------------------------------------

def _build_prog(chs, npad, bd, phase2):
    """One SPMD program over wpc = len(chs) window slots; slot j has chs[j]
    128-edge chunks.  phase2 selects the combine epilogue."""
    from concourse import bacc, tile, library_config
    import concourse.mybir as mybir

    f32 = mybir.dt.float32
    bf16 = mybir.dt.bfloat16
    i16 = mybir.dt.int16
    eq = mybir.AluOpType.is_equal
    mul = mybir.AluOpType.mult
    add = mybir.AluOpType.add
    copy_f = mybir.ActivationFunctionType.Copy

    wpc = len(chs)
    CH = int(sum(chs))
    off = np.concatenate([[0], np.cumsum(chs)]).astype(int)

    nc = bacc.Bacc(
        "TRN2",
        target_bir_lowering=False,
        debug=False,
        num_devices=NC_CORES,
        num_swdge_queues=4,
    )

    srcg = nc.dram_tensor("srcg", [npad, bd], bf16, kind="ExternalInput")
    idx_d = nc.dram_tensor("idx", [128, CH * 8], i16, kind="ExternalInput")
    dst_d = nc.dram_tensor("dstl", [128, CH], f32, kind="ExternalInput")
    nra_d = nc.dram_tensor("nra", [128, CH], f32, kind="ExternalInput")
    iota_d = nc.dram_tensor("iota", [128, 128], bf16, kind="ExternalInput")
    ident_d = nc.dram_tensor("ident", [128, 128], bf16, kind="ExternalInput")
    if phase2:
        outp_d = nc.dram_tensor("outp", [wpc, 64, 1024], bf16, kind="ExternalInput")
        xt_d = nc.dram_tensor("xt", [wpc, 64, 1024], bf16, kind="ExternalInput")
        w02_d = nc.dram_tensor("w02", [64, 64], bf16, kind="ExternalInput")
        outt_d = nc.dram_tensor("outt", [wpc, 64, 1024], bf16, kind="ExternalOutput")
    else:
        w1_d = nc.dram_tensor("w1", [64, 64], bf16, kind="ExternalInput")
        w2_d = nc.dram_tensor("w2", [64, 64], bf16, kind="ExternalInput")
        bias_d = nc.dram_tensor("bias", [64, 1], f32, kind="ExternalInput")
        zt_d = nc.dram_tensor("zt", [wpc, 64, 1024], bf16, kind="ExternalOutput")
        outp_d = nc.dram_tensor("outp", [wpc, 64, 1024], bf16, kind="ExternalOutput")

    with tile.TileContext(nc) as tc:
        nc.gpsimd.load_library(library_config.mlp)
        with (
            tc.tile_pool(name="const", bufs=1) as constp,
            tc.tile_pool(name="gat", bufs=10) as gatp,
            tc.tile_pool(name="meta", bufs=6) as metap,
            tc.tile_pool(name="oh", bufs=5) as ohp,
            tc.tile_pool(name="sb", bufs=3) as sbp,
            tc.tile_pool(name="out", bufs=3) as outp_pool,
            tc.tile_pool(name="ps", bufs=3, space="PSUM") as psp,
            tc.tile_pool(name="tps", bufs=2 if phase2 else 1, space="PSUM") as tpsp,
            tc.tile_pool(name="ops", bufs=1, space="PSUM") as opsp,
        ):
            gq = [0]  # global gather-call counter for queue round-robin
            iota_t = constp.tile([128, 128], bf16, tag="iota")
            nc.sync.dma_start(iota_t[:], iota_d[:])
            ident_t = constp.tile([128, 128], bf16, tag="ident")
            nc.sync.dma_start(ident_t[:], ident_d[:])
            if phase2:
                w02_t = constp.tile([64, 64], bf16, tag="w02")
                nc.sync.dma_start(w02_t[:], w02_d[:])
            else:
                w1_t = constp.tile([64, 64], bf16, tag="w1")
                nc.sync.dma_start(w1_t[:], w1_d[:])
                w2_t = constp.tile([64, 64], bf16, tag="w2")
                nc.sync.dma_start(w2_t[:], w2_d[:])
                bias_t = constp.tile([64, 1], f32, tag="bias")
                nc.sync.dma_start(bias_t[:], bias_d[:])

            for j in range(wpc):
                ch = int(chs[j])
                c0, c1 = int(off[j]), int(off[j + 1])
                idx_t = metap.tile([128, ch * 8], i16, tag="idx")
                nc.sync.dma_start(idx_t[:], idx_d[:, c0 * 8 : c1 * 8])
                dst_t = metap.tile([128, ch], f32, tag="dst")
                nc.sync.dma_start(dst_t[:], dst_d[:, c0:c1])
                nra_t = metap.tile([128, ch], f32, tag="nra")
                nc.sync.dma_start(nra_t[:], nra_d[:, c0:c1])
                if phase2:
                    outp_t = outp_pool.tile([64, 1024], bf16, tag="outp")
                    nc.sync.dma_start(outp_t[:], outp_d[j])
                    xt_t = outp_pool.tile([64, 1024], bf16, tag="xt")
                    nc.sync.dma_start(xt_t[:], xt_d[j])

                # One-hot scatter matrices for the whole window, two batched
                # DVE ops: S[p, c, f] = (iota[f] == dst[p, c]) * |nrm[p, c]|.
                # Sign of norm folded into the epilogue scales.
                s_all = ohp.tile([128, ch, 128], bf16, tag="s")
                iota_b = (
                    iota_t[:]
                    .rearrange("p (o f) -> p o f", o=1)
                    .broadcast_to([128, ch, 128])
                )
                dst_b = (
                    dst_t[:]
                    .rearrange("p (c o) -> p c o", o=1)
                    .broadcast_to([128, ch, 128])
                )
                nra_b = (
                    nra_t[:]
                    .rearrange("p (c o) -> p c o", o=1)
                    .broadcast_to([128, ch, 128])
                )
                nc.vector.tensor_tensor(s_all[:], iota_b, dst_b, op=eq)
                nc.vector.tensor_tensor(s_all[:], s_all[:], nra_b, op=mul)

                # per-edge source rows via SWDGE gather, balanced calls
                ncalls = -(-ch // GSEG)
                base, rem = divmod(ch, ncalls)
                segs = [base + (k < rem) for k in range(ncalls)]
                g_ts = []
                s0 = 0
                for seg in segs:
                    g_t = gatp.tile([128, GSEG, bd], bf16, tag="g")
                    nc.gpsimd.dma_gather(
                        g_t[:, :seg, :],
                        srcg.ap(),
                        idx_t[:, s0 * 8 : (s0 + seg) * 8],
                        seg * 128,
                        seg * 128,
                        bd,
                        queue_num=gq[0] % 4,
                    )
                    gq[0] += 1
                    g_ts.append((g_t, s0, seg))
                    s0 += seg

                # psum[dst, bf] -= sum_e norm_e * src_row_e  (S holds |norm|)
                ps = psp.tile([128, bd], f32, tag="acc")
                c = 0
                for g_t, s0, seg in g_ts:
                    for cc in range(seg):
                        nc.tensor.matmul(
                            ps[:],
                            s_all[:, c, :],
                            g_t[:, cc, :],
                            start=(c == 0),
                            stop=(c == ch - 1),
                        )
                        c += 1

                # h_sb = scale * psum  (scale -1 -> Tx1;  -2 -> 2*P(z))
                h_sb = sbp.tile([128, bd], bf16, tag="h")
                nc.scalar.activation(
                    h_sb[:], ps[:], copy_f, scale=-2.0 if phase2 else -1.0
                )
                # 8 transposes -> tps[64, 1024] = h^T
                tps = tpsp.tile([64, 1024], bf16, tag="tp")
                for b in range(8):
                    nc.tensor.transpose(
                        tps[:, b * 128 : (b + 1) * 128],
                        h_sb[:, b * 64 : (b + 1) * 64],
                        ident_t[:],
                    )

                if phase2:
                    # cps = (W0-W2)^T x^T
                    cps = opsp.tile([64, 1024], f32, tag="cps")
                    for q in range(2):
                        nc.tensor.matmul(
                            cps[:, q * 512 : (q + 1) * 512],
                            w02_t[:],
                            xt_t[:, q * 512 : (q + 1) * 512],
                            start=True,
                            stop=True,
                        )
                    # out^T = outP + 2*P(z)^T + (W0-W2)^T x^T
                    o_sb = outp_pool.tile([64, 1024], bf16, tag="o")
                    nc.vector.tensor_tensor(o_sb[:], tps[:], outp_t[:], op=add)
                    nc.vector.tensor_tensor(o_sb[:], o_sb[:], cps[:], op=add)
                    nc.sync.dma_start(outt_d[j], o_sb[:])
                else:
                    t1t = sbp.tile([64, 1024], bf16, tag="t1t")
                    nc.scalar.copy(t1t[:], tps[:])
                    # zT = W2^T Tx1^T
                    zps = opsp.tile([64, 1024], f32, tag="zps")
                    for q in range(2):
                        nc.tensor.matmul(
                            zps[:, q * 512 : (q + 1) * 512],
                            w2_t[:],
                            t1t[:, q * 512 : (q + 1) * 512],
                            start=True,
                            stop=True,
                        )
                    z_sb = sbp.tile([64, 1024], bf16, tag="z")
                    nc.scalar.copy(z_sb[:], zps[:])
                    nc.sync.dma_start(zt_d[j], z_sb[:])
                    # outP = W1^T Tx1^T + bias
                    ops = opsp.tile([64, 1024], f32, tag="ops")
                    for q in range(2):
                        nc.tensor.matmul(
                            ops[:, q * 512 : (q + 1) * 512],
                            w1_t[:],
                            t1t[:, q * 512 : (q + 1) * 512],
                            start=True,
                            stop=True,
                        )
                    op_sb = outp_pool.tile([64, 1024], bf16, tag="opsb")
                    nc.vector.tensor_scalar(
                        op_sb[:], ops[:], bias_t[:, 0:1], None, op0=add
                    )
                    nc.sync.dma_start(outp_d[j], op_sb[:])
    nc.compile()
    return nc


# ----------------------------------------------------------------------------
# entry point
# ----------------------------------------------------------------------------

LAST_EXEC_NS = []
_LAUNCH_NO = [0]


def _launch(nc, in_maps, trace):
    from concourse.bass_utils import run_bass_kernel_spmd

    tmpdir = None
    base = os.environ.get("CHEB_TMPDIR")
    if base:
        _LAUNCH_NO[0] += 1
        tmpdir = os.path.join(base, f"l{_LAUNCH_NO[0]}")
        os.makedirs(tmpdir, exist_ok=True)
    return run_bass_kernel_spmd(
        nc, in_maps, list(range(len(in_maps))), trace=trace, tmpdir=tmpdir
    )


def kernel(x, edge_index, edge_attr, W, bias):
    import ml_dtypes

    bf = ml_dtypes.bfloat16
    trace = bool(int(os.environ.get("CHEB_TRACE", "0")))

    B, N, D = x.shape
    bd = B * D
    nw = -(-N // NPW)  # windows for real nodes
    nw = -(-nw // NC_CORES) * NC_CORES  # pad to multiple of cores
    wpc = nw // NC_CORES
    npad = nw * NPW

    cnt, srt_row, srt_col, srt_norm = _prep_edges(edge_index, edge_attr, N, nw)
    pos = np.concatenate([[0], np.cumsum(cnt)]).astype(int)

    # window -> (slot, core) by descending edge count: slot j's chunk count is
    # shared across cores with minimal padding.
    order = np.argsort(-cnt, kind="stable")
    wins = order.reshape(wpc, NC_CORES)  # wins[j, c] = window of core c, slot j
    chs = [int(-(-cnt[wins[j, 0]] // 128)) for j in range(wpc)]
    CH = int(sum(chs))
    off = np.concatenate([[0], np.cumsum(chs)]).astype(int)

    # per-(core, slot) metadata, flattened along chunks
    src_pad = np.zeros((NC_CORES, CH * 128), np.int16)
    dstl_pad = np.zeros((NC_CORES, CH * 128), np.float32)
    norm_pad = np.zeros((NC_CORES, CH * 128), np.float32)
    for j in range(wpc):
        for c in range(NC_CORES):
            w = int(wins[j, c])
            e0, e1 = int(pos[w]), int(pos[w + 1])
            k = e1 - e0
            s0 = int(off[j]) * 128
            src_pad[c, s0 : s0 + k] = srt_row[e0:e1]
            dstl_pad[c, s0 : s0 + k] = (srt_col[e0:e1] - w * NPW).astype(np.float32)
            norm_pad[c, s0 : s0 + k] = srt_norm[e0:e1]

    idx_all = _wrap16(src_pad)  # [cores, 128, CH*8] int16
    dst_all = _wrap128(dstl_pad)  # [cores, 128, CH]
    nra_all = -_wrap128(norm_pad)  # |norm| (norm <= 0)

    iota = np.broadcast_to(np.arange(128, dtype=np.float32), (128, 128)).astype(bf)
    ident = np.eye(128, dtype=np.float32).astype(bf)

    # gather table for launch 1: node-major, all batches contiguous
    xg = np.zeros((npad, bd), bf)
    xnb = np.ascontiguousarray(x.transpose(1, 0, 2)).reshape(N, bd)
    xg[:N] = xnb.astype(bf)

    # x^T tiles per (core, slot): [64, b*128+nl]
    xpad = np.zeros((B, npad, D), np.float32)
    xpad[:, :N] = x
    # xt_full[w, d, b, nl] = x[b, w*128+nl, d]
    xt_full = xpad.reshape(B, nw, NPW, D).transpose(1, 3, 0, 2).astype(bf)
    xt_full = np.ascontiguousarray(xt_full.reshape(nw, 64, 1024))

    W = W.astype(np.float32)
    w1 = np.ascontiguousarray(W[1]).astype(bf)
    w2 = np.ascontiguousarray(W[2]).astype(bf)
    w02 = np.ascontiguousarray(W[0] - W[2]).astype(bf)
    bias_in = bias.astype(np.float32).reshape(64, 1)

    core_ids = list(range(NC_CORES))

    # ---- launch 1 ----
    prog1 = _build_prog(chs, npad, bd, phase2=False)
    in_maps1 = []
    for c in core_ids:
        in_maps1.append(
            {
                "srcg": xg,
                "idx": np.ascontiguousarray(idx_all[c]),
                "dstl": np.ascontiguousarray(dst_all[c]),
                "nra": np.ascontiguousarray(nra_all[c]),
                "iota": iota,
                "ident": ident,
                "w1": w1,
                "w2": w2,
                "bias": bias_in,
            }
        )
    r1 = _launch(prog1, in_maps1, trace)

    # assemble z table (node-major) from zT tiles; keep outP per core
    zg = np.zeros((npad, bd), bf)
    outp_tiles = []
    for c in core_ids:
        zt = r1.results[c]["zt"]  # [wpc, 64, 1024] bf16
        outp_tiles.append(r1.results[c]["outp"])
        # z[w*128+nl, b*64+d] = zt[j, d, b*128+nl]
        z = zt.reshape(wpc, 64, 8, 128).transpose(0, 3, 2, 1)  # [j, nl, b, d]
        zg[(wins[:, c][:, None] * NPW + np.arange(NPW)[None, :]).reshape(-1)] = (
            z.reshape(wpc * NPW, bd)
        )

    # ---- launch 2 ----
    prog2 = _build_prog(chs, npad, bd, phase2=True)
    in_maps2 = []
    for c in core_ids:
        in_maps2.append(
            {
                "srcg": zg,
                "idx": np.ascontiguousarray(idx_all[c]),
                "dstl": np.ascontiguousarray(dst_all[c]),
                "nra": np.ascontiguousarray(nra_all[c]),
                "iota": iota,
                "ident": ident,
                "outp": outp_tiles[c],
                "xt": np.ascontiguousarray(xt_full[wins[:, c]]),
                "w02": w02,
            }
        )
    r2 = _launch(prog2, in_maps2, trace)

    global LAST_EXEC_NS
    LAST_EXEC_NS = [r1.exec_time_ns, r2.exec_time_ns]

    # out[b, w*128+nl, e] = outt[c][j, e, b*128+nl]
    out = np.empty((B, npad, 64), np.float32)
    for c in core_ids:
        ot = r2.results[c]["outt"].astype(np.float32)  # [wpc, 64, 1024]
        ot = ot.reshape(wpc, 64, 8, 128).transpose(2, 0, 3, 1)  # [b, j, nl, e]
        w_ids = wins[:, c]
        out[:, (w_ids[:, None] * NPW + np.arange(NPW)[None, :]).reshape(-1), :] = (
            ot.reshape(B, wpc * NPW, 64)
        )
    return out[:, :N, :]


# revision 18
# speedup vs baseline: 1.4824x; 1.0902x over previous
"""Batched ChebConv (K=3) Trainium2 kernel.

Strategy (dst-node sharding, 8 cores, 2 launches):
  out = x@W0 + Tx1@W1 + Tx2@W2,  Tx1 = P(x),  Tx2 = 2*P(Tx1) - x
      = x@(W0-W2) + Tx1@W1 + 2*P(Tx1@W2)        [P commutes with W]

  Feature math runs in the transposed domain (features in partitions):
    out^T = (W0-W2)^T x^T + W1^T Tx1^T + 2*P(z)^T,   z = Tx1@W2.

  Launch 1: per dst window, scatter-matmul propagation psum = -P(x), then
    8 PE transposes of Tx1, zT = W2^T Tx1^T and outP = W1^T Tx1^T + bias.
    Host relayouts zT -> node-major z gather table between launches.
  Launch 2: propagation on z, cps = (W0-W2)^T x^T,
    out^T = outP + 2*P(z)^T + cps.

  Propagation: edges grouped by dst window; per window the DISTINCT source
  nodes are fetched once (SWDGE dma_gather, 4 queues round-robin, int16
  idxs, -1 tail padding so the ucode trims).  Sources are sorted by edge
  multiplicity (desc) and chunked by 128; chunk c needs npass_c = max
  multiplicity passes.  Pass t gets a one-hot matrix S_t[src_lane, dst] =
  |norm| of that source's t-th edge (fused DVE tensor_scalar vs iota), and
  PE accumulates psum += S_t^T @ gathered_chunk.

  Windows are assigned to (core, slot) by descending edge count so slot
  shapes (gather chunks, pass counts) are shared across cores (SPMD) with
  minimal padding.
"""

import os
import numpy as np

NC_CORES = 8
NPW = 128  # nodes per window
GSEG = 8  # max chunks per dma_gather call (1024 idxs; HW fails above ~1k)


# ----------------------------------------------------------------------------
# host-side prep
# ----------------------------------------------------------------------------

def _prep_edges(edge_index, edge_attr, n_nodes, n_windows):
    """Sort edges by destination window, then source.  Returns per-window
    counts and the sorted row/col/|norm| arrays."""
    row = edge_index[0].astype(np.int64)
    col = edge_index[1].astype(np.int64)
    ea = edge_attr.astype(np.float64)

    deg = np.zeros(n_nodes, np.float64)
    np.add.at(deg, row, ea)
    deg = deg.astype(np.float32)
    dis = np.where(deg > 0, 1.0 / np.sqrt(deg), 0.0).astype(np.float32)
    nra = dis[row] * edge_attr.astype(np.float32) * dis[col]  # = -norm >= 0

    w_of_edge = col // NPW
    order = np.lexsort((row, w_of_edge))
    cnt = np.bincount(w_of_edge, minlength=n_windows)
    return cnt, row[order], col[order], nra[order]


def _wrap16(a):
    """Element i -> [i%16, i//16], replicated to 128 partitions."""
    n = a.shape[-1]
    w = a.reshape(*a.shape[:-1], n // 16, 16)
    w = np.swapaxes(w, -1, -2)  # [..., 16, n//16]
    return np.concatenate([w] * 8, axis=-2)  # [..., 128, n//16]


# ----------------------------------------------------------------------------
# device program
# ----------------------------------------------------------------------------

def _build_prog(slots, npad, bd, phase2):
    """slots: list of (gch, pcs) per window slot — gch gather chunks and
    pcs[t] = chunk index of pass-slot t.  phase2 selects the epilogue."""
    from concourse import bacc, tile, library_config
    import concourse.mybir as mybir

    f32 = mybir.dt.float32
    bf16 = mybir.dt.bfloat16
    i16 = mybir.dt.int16
    eq = mybir.AluOpType.is_equal
    mul = mybir.AluOpType.mult
    add = mybir.AluOpType.add
    copy_f = mybir.ActivationFunctionType.Copy

    wpc = len(slots)
    GT = int(sum(g for g, _ in slots))  # total gather chunks
    PT = int(sum(len(p) for _, p in slots))  # total pass slots
    goff = np.concatenate([[0], np.cumsum([g for g, _ in slots])]).astype(int)
    poff = np.concatenate([[0], np.cumsum([len(p) for _, p in slots])]).astype(int)

    nc = bacc.Bacc(
        "TRN2",
        target_bir_lowering=False,
        debug=False,
        num_devices=NC_CORES,
        num_swdge_queues=4,
    )

    srcg = nc.dram_tensor("srcg", [npad, bd], bf16, kind="ExternalInput")
    idx_d = nc.dram_tensor("idx", [128, GT * 8], i16, kind="ExternalInput")
    dst_d = nc.dram_tensor("dstl", [128, PT], f32, kind="ExternalInput")
    nra_d = nc.dram_tensor("nra", [128, PT], f32, kind="ExternalInput")
    iota_d = nc.dram_tensor("iota", [128, 128], bf16, kind="ExternalInput")
    ident_d = nc.dram_tensor("ident", [128, 128], bf16, kind="ExternalInput")
    if phase2:
        outp_d = nc.dram_tensor("outp", [wpc, 64, 1024], bf16, kind="ExternalInput")
        xt_d = nc.dram_tensor("xt", [wpc, 64, 1024], bf16, kind="ExternalInput")
        w02_d = nc.dram_tensor("w02", [64, 64], bf16, kind="ExternalInput")
        outt_d = nc.dram_tensor("outt", [wpc, 64, 1024], bf16, kind="ExternalOutput")
    else:
        w1_d = nc.dram_tensor("w1", [64, 64], bf16, kind="ExternalInput")
        w2_d = nc.dram_tensor("w2", [64, 64], bf16, kind="ExternalInput")
        bias_d = nc.dram_tensor("bias", [64, 1], f32, kind="ExternalInput")
        zt_d = nc.dram_tensor("zt", [wpc, 64, 1024], bf16, kind="ExternalOutput")
        outp_d = nc.dram_tensor("outp", [wpc, 64, 1024], bf16, kind="ExternalOutput")

    with tile.TileContext(nc) as tc:
        nc.gpsimd.load_library(library_config.mlp)
        with (
            tc.tile_pool(name="const", bufs=1) as constp,
            tc.tile_pool(name="gat", bufs=10) as gatp,
            tc.tile_pool(name="meta", bufs=6) as metap,
            tc.tile_pool(name="oh", bufs=5) as ohp,
            tc.tile_pool(name="sb", bufs=3) as sbp,
            tc.tile_pool(name="out", bufs=3) as outp_pool,
            tc.tile_pool(name="ps", bufs=3, space="PSUM") as psp,
            tc.tile_pool(name="tps", bufs=2 if phase2 else 1, space="PSUM") as tpsp,
            tc.tile_pool(name="ops", bufs=1, space="PSUM") as opsp,
        ):
            gq = [0]  # global gather-call counter for queue round-robin
            iota_t = constp.tile([128, 128], bf16, tag="iota")
            nc.sync.dma_start(iota_t[:], iota_d[:])
            ident_t = constp.tile([128, 128], bf16, tag="ident")
            nc.sync.dma_start(ident_t[:], ident_d[:])
            if phase2:
                w02_t = constp.tile([64, 64], bf16, tag="w02")
                nc.sync.dma_start(w02_t[:], w02_d[:])
            else:
                w1_t = constp.tile([64, 64], bf16, tag="w1")
                nc.sync.dma_start(w1_t[:], w1_d[:])
                w2_t = constp.tile([64, 64], bf16, tag="w2")
                nc.sync.dma_start(w2_t[:], w2_d[:])
                bias_t = constp.tile([64, 1], f32, tag="bias")
                nc.sync.dma_start(bias_t[:], bias_d[:])

            for j in range(wpc):
                gch, pcs = slots[j]
                ps_n = len(pcs)
                g0, p0 = int(goff[j]), int(poff[j])
                idx_t = metap.tile([128, gch * 8], i16, tag="idx")
                nc.sync.dma_start(idx_t[:], idx_d[:, g0 * 8 : (g0 + gch) * 8])
                dst_t = metap.tile([128, ps_n], f32, tag="dst")
                nc.sync.dma_start(dst_t[:], dst_d[:, p0 : p0 + ps_n])
                nra_t = metap.tile([128, ps_n], f32, tag="nra")
                nc.sync.dma_start(nra_t[:], nra_d[:, p0 : p0 + ps_n])
                if phase2:
                    outp_t = outp_pool.tile([64, 1024], bf16, tag="outp")
                    nc.sync.dma_start(outp_t[:], outp_d[j])
                    xt_t = outp_pool.tile([64, 1024], bf16, tag="xt")
                    nc.sync.dma_start(xt_t[:], xt_d[j])

                # distinct-source rows via SWDGE gather, balanced calls
                ncalls = -(-gch // GSEG)
                base, rem = divmod(gch, ncalls)
                segs = [base + (k < rem) for k in range(ncalls)]
                g_ts = []  # (tile, within-call chunk) per global chunk
                s0 = 0
                for seg in segs:
                    g_t = gatp.tile([128, GSEG, bd], bf16, tag="g")
                    nc.gpsimd.dma_gather(
                        g_t[:, :seg, :],
                        srcg.ap(),
                        idx_t[:, s0 * 8 : (s0 + seg) * 8],
                        seg * 128,
                        seg * 128,
                        bd,
                        queue_num=gq[0] % 4,
                    )
                    gq[0] += 1
                    for cc in range(seg):
                        g_ts.append((g_t, cc))
                    s0 += seg

                # per pass-slot one-hot matrices, two batched DVE ops:
                # S[p, t, f] = (iota[f] == dst[p, t]) * |nrm[p, t]|
                s_all = ohp.tile([128, ps_n, 128], bf16, tag="s")
                ps = psp.tile([128, bd], f32, tag="acc")
                iota_b = (
                    iota_t[:]
                    .rearrange("p (o f) -> p o f", o=1)
                    .broadcast_to([128, ps_n, 128])
                )
                dst_b = (
                    dst_t[:]
                    .rearrange("p (c o) -> p c o", o=1)
                    .broadcast_to([128, ps_n, 128])
                )
                nra_b = (
                    nra_t[:]
                    .rearrange("p (c o) -> p c o", o=1)
                    .broadcast_to([128, ps_n, 128])
                )
                nc.vector.tensor_tensor(s_all[:], iota_b, dst_b, op=eq)
                nc.vector.tensor_tensor(s_all[:], s_all[:], nra_b, op=mul)
                for t in range(ps_n):
                    g_t, cc = g_ts[pcs[t]]
                    nc.tensor.matmul(
                        ps[:],
                        s_all[:, t, :],
                        g_t[:, cc, :],
                        start=(t == 0),
                        stop=(t == ps_n - 1),
                    )

                # h_sb = scale * psum  (scale -1 -> Tx1;  -2 -> 2*P(z))
                h_sb = sbp.tile([128, bd], bf16, tag="h")
                nc.scalar.activation(
                    h_sb[:], ps[:], copy_f, scale=-2.0 if phase2 else -1.0
                )
                # 8 transposes -> tps[64, 1024] = h^T
                tps = tpsp.tile([64, 1024], bf16, tag="tp")
                for b in range(8):
                    nc.tensor.transpose(
                        tps[:, b * 128 : (b + 1) * 128],
                        h_sb[:, b * 64 : (b + 1) * 64],
                        ident_t[:],
                    )

                if phase2:
                    # cps = (W0-W2)^T x^T
                    cps = opsp.tile([64, 1024], f32, tag="cps")
                    for q in range(2):
                        nc.tensor.matmul(
                            cps[:, q * 512 : (q + 1) * 512],
                            w02_t[:],
                            xt_t[:, q * 512 : (q + 1) * 512],
                            start=True,
                            stop=True,
                        )
                    # out^T = outP + 2*P(z)^T + (W0-W2)^T x^T
                    o_sb = outp_pool.tile([64, 1024], bf16, tag="o")
                    nc.vector.tensor_tensor(o_sb[:], tps[:], outp_t[:], op=add)
                    nc.vector.tensor_tensor(o_sb[:], o_sb[:], cps[:], op=add)
                    nc.sync.dma_start(outt_d[j], o_sb[:])
                else:
                    t1t = sbp.tile([64, 1024], bf16, tag="t1t")
                    nc.scalar.copy(t1t[:], tps[:])
                    # zT = W2^T Tx1^T
                    zps = opsp.tile([64, 1024], f32, tag="zps")
                    for q in range(2):
                        nc.tensor.matmul(
                            zps[:, q * 512 : (q + 1) * 512],
                            w2_t[:],
                            t1t[:, q * 512 : (q + 1) * 512],
                            start=True,
                            stop=True,
                        )
                    z_sb = sbp.tile([64, 1024], bf16, tag="z")
                    nc.scalar.copy(z_sb[:], zps[:])
                    nc.sync.dma_start(zt_d[j], z_sb[:])
                    # outP = W1^T Tx1^T + bias
                    ops = opsp.tile([64, 1024], f32, tag="ops")
                    for q in range(2):
                        nc.tensor.matmul(
                            ops[:, q * 512 : (q + 1) * 512],
                            w1_t[:],
                            t1t[:, q * 512 : (q + 1) * 512],
                            start=True,
                            stop=True,
                        )
                    op_sb = outp_pool.tile([64, 1024], bf16, tag="opsb")
                    nc.vector.tensor_scalar(
                        op_sb[:], ops[:], bias_t[:, 0:1], None, op0=add
                    )
                    nc.sync.dma_start(outp_d[j], op_sb[:])
    nc.compile()
    return nc


# ----------------------------------------------------------------------------
# entry point
# ----------------------------------------------------------------------------

LAST_EXEC_NS = []
_LAUNCH_NO = [0]


def _launch(nc, in_maps, trace):
    from concourse.bass_utils import run_bass_kernel_spmd

    tmpdir = None
    base = os.environ.get("CHEB_TMPDIR")
    if base:
        _LAUNCH_NO[0] += 1
        tmpdir = os.path.join(base, f"l{_LAUNCH_NO[0]}")
        os.makedirs(tmpdir, exist_ok=True)
    return run_bass_kernel_spmd(
        nc, in_maps, list(range(len(in_maps))), trace=trace, tmpdir=tmpdir
    )


def kernel(x, edge_index, edge_attr, W, bias):
    import ml_dtypes

    bf = ml_dtypes.bfloat16
    trace = bool(int(os.environ.get("CHEB_TRACE", "0")))

    B, N, D = x.shape
    bd = B * D
    nw = -(-N // NPW)
    nw = -(-nw // NC_CORES) * NC_CORES
    wpc = nw // NC_CORES
    npad = nw * NPW

    cnt, srt_row, srt_col, srt_nra = _prep_edges(edge_index, edge_attr, N, nw)
    pos = np.concatenate([[0], np.cumsum(cnt)]).astype(int)

    # window -> (slot, core) by descending edge count
    order = np.argsort(-cnt, kind="stable")
    wins = order.reshape(wpc, NC_CORES)

    # per-window dedup: distinct sources sorted by multiplicity desc
    dedup = {}
    for w in range(nw):
        sl = slice(int(pos[w]), int(pos[w + 1]))
        srcs, first, counts = np.unique(
            srt_row[sl], return_index=True, return_counts=True
        )
        o = np.argsort(-counts, kind="stable")
        dedup[w] = (srcs[o], first[o], counts[o], sl)

    # shared slot shapes: gather chunks + per-chunk pass counts (max over
    # the 8 cores in the slot)
    slots = []
    for j in range(wpc):
        gch = max(-(-len(dedup[wins[j, c]][0]) // 128) for c in range(NC_CORES))
        gch = max(gch, 1)
        npass = np.zeros(gch, np.int64)
        for c in range(NC_CORES):
            counts = dedup[wins[j, c]][2]
            for ck in range(-(-len(counts) // 128)):
                npass[ck] = max(npass[ck], counts[ck * 128])
        npass = np.maximum(npass, 0)
        pcs = []
        for ck in range(gch):
            pcs.extend([ck] * int(max(npass[ck], 1)))
        slots.append((int(gch), tuple(pcs)))

    GT = int(sum(g for g, _ in slots))
    PT = int(sum(len(p) for _, p in slots))
    goff = np.concatenate([[0], np.cumsum([g for g, _ in slots])]).astype(int)
    poff = np.concatenate([[0], np.cumsum([len(p) for _, p in slots])]).astype(int)

    # chunk-local pass offsets per slot: pass-slot index of (chunk, k)
    cpoff = []
    for gch, pcs in slots:
        co = np.zeros(gch, np.int64)
        arr = np.asarray(pcs)
        for ck in range(gch):
            idxs = np.nonzero(arr == ck)[0]
            co[ck] = idxs[0]
        cpoff.append(co)

    src_flat = np.zeros((NC_CORES, GT * 128), np.int16)
    dstp = np.zeros((NC_CORES, 128, PT), np.float32)
    nrap = np.zeros((NC_CORES, 128, PT), np.float32)
    for j in range(wpc):
        g0, p0 = int(goff[j]), int(poff[j])
        for c in range(NC_CORES):
            w = int(wins[j, c])
            srcs, first, counts, sl = dedup[w]
            m = len(srcs)
            if m == 0:
                continue
            src_flat[c, g0 * 128 : g0 * 128 + m] = srcs.astype(np.int16)
            cols_l = (srt_col[sl] - w * NPW).astype(np.float32)
            nras = srt_nra[sl].astype(np.float32)
            reps = counts
            tot = int(reps.sum())
            r_ids = np.repeat(np.arange(m), reps)
            k_ids = np.arange(tot) - np.repeat(np.cumsum(reps) - reps, reps)
            e_ids = np.repeat(first, reps) + k_ids
            lanes = r_ids % 128
            t_ids = cpoff[j][r_ids // 128] + k_ids
            dstp[c, lanes, p0 + t_ids] = cols_l[e_ids]
            nrap[c, lanes, p0 + t_ids] = nras[e_ids]

    idx_all = _wrap16(src_flat)  # [cores, 128, GT*8] int16

    iota = np.broadcast_to(np.arange(128, dtype=np.float32), (128, 128)).astype(bf)
    ident = np.eye(128, dtype=np.float32).astype(bf)

    # gather table for launch 1: node-major, all batches contiguous
    xg = np.zeros((npad, bd), bf)
    xg[:N] = np.ascontiguousarray(x.transpose(1, 0, 2)).reshape(N, bd).astype(bf)

    # x^T tiles per window: [64, b*128+nl]
    xpad = np.zeros((B, npad, D), np.float32)
    xpad[:, :N] = x
    xt_full = xpad.reshape(B, nw, NPW, D).transpose(1, 3, 0, 2).astype(bf)
    xt_full = np.ascontiguousarray(xt_full.reshape(nw, 64, 1024))

    W = W.astype(np.float32)
    w1 = np.ascontiguousarray(W[1]).astype(bf)
    w2 = np.ascontiguousarray(W[2]).astype(bf)
    w02 = np.ascontiguousarray(W[0] - W[2]).astype(bf)
    bias_in = bias.astype(np.float32).reshape(64, 1)

    core_ids = list(range(NC_CORES))

    # ---- launch 1 ----
    prog1 = _build_prog(slots, npad, bd, phase2=False)
    in_maps1 = []
    for c in core_ids:
        in_maps1.append(
            {
                "srcg": xg,
                "idx": np.ascontiguousarray(idx_all[c]),
                "dstl": np.ascontiguousarray(dstp[c]),
                "nra": np.ascontiguousarray(nrap[c]),
                "iota": iota,
                "ident": ident,
                "w1": w1,
                "w2": w2,
                "bias": bias_in,
            }
        )
    r1 = _launch(prog1, in_maps1, trace)

    # assemble z table (node-major) from zT tiles; keep outP per core
    zg = np.zeros((npad, bd), bf)
    outp_tiles = []
    for c in core_ids:
        zt = r1.results[c]["zt"]  # [wpc, 64, 1024] bf16
        outp_tiles.append(r1.results[c]["outp"])
        z = zt.reshape(wpc, 64, 8, 128).transpose(0, 3, 2, 1)  # [j, nl, b, d]
        zg[(wins[:, c][:, None] * NPW + np.arange(NPW)[None, :]).reshape(-1)] = (
            z.reshape(wpc * NPW, bd)
        )

    # ---- launch 2 ----
    prog2 = _build_prog(slots, npad, bd, phase2=True)
    in_maps2 = []
    for c in core_ids:
        in_maps2.append(
            {
                "srcg": zg,
                "idx": np.ascontiguousarray(idx_all[c]),
                "dstl": np.ascontiguousarray(dstp[c]),
                "nra": np.ascontiguousarray(nrap[c]),
                "iota": iota,
                "ident": ident,
                "outp": outp_tiles[c],
                "xt": np.ascontiguousarray(xt_full[wins[:, c]]),
                "w02": w02,
            }
        )
    r2 = _launch(prog2, in_maps2, trace)

    global LAST_EXEC_NS
    LAST_EXEC_NS = [r1.exec_time_ns, r2.exec_time_ns]

    # out[b, w*128+nl, e] = outt[c][j, e, b*128+nl]
    out = np.empty((B, npad, 64), np.float32)
    for c in core_ids:
        ot = r2.results[c]["outt"].astype(np.float32)
        ot = ot.reshape(wpc, 64, 8, 128).transpose(2, 0, 3, 1)
        w_ids = wins[:, c]
        out[:, (w_ids[:, None] * NPW + np.arange(NPW)[None, :]).reshape(-1), :] = (
            ot.reshape(B, wpc * NPW, 64)
        )
    return out[:, :N, :]
